# revision 36
# baseline (speedup 1.0000x reference)
"""Multi-head attention block (B=4, N=2048, D=1024, H=16) on 8 trn2 NeuronCores.

Sharding: core c -> (batch b = c//2, head-group g = c%2) with 8 heads per
group.  Each core computes q/k/v for its 8 heads over its batch, full
attention, and a partial projection y_part = attn_out_g @ w_proj[rows_g].
Host combines: out[b] = y_part[2b] + y_part[2b+1] + b_proj.

All device compute is bf16 (inputs converted host-side, including x
pre-transposed to x.T so the PE array never transposes activations).
q'/k' stay resident in SBUF as pair-stacked [128, seq] tiles; the odd
head's 64-partition half is rebased to partition 0 with an SBUF->SBUF DMA
(engines can't cross partitions; DMA can).

Attention runs "swapped": scores land as [keys, q] strips, exp'd p chunks
become the stationary matmul operand so PV emits [q, head_dim] tiles —
the softmax denominator (a ones-column of v_aug) is then a per-partition
scalar handled by reciprocal + tensor_scalar, no broadcast needed.  The
four PV accumulation groups share one PSUM bank: a single start=True
zeroes the whole 2KB zero-region.  Normalized [q, hd] pairs are
transposed back to [hd, q] pair-stacked ostack form by the DMA XBAR.

Emission keeps both PE and ACT dense: a minimal prologue (one k window +
one q window), everything else (rest of pair-0, v tiles, later pairs'
q/k, projection row-tiles) flows through a deadline-ordered background
queue drained lazily between attention strips; PV batches and unit
epilogues trail by one strip / one unit so nothing waits on the freshest
exp.
"""
import sys

sys.path.insert(0, "/opt/trn_rl_repo")

import numpy as np

import concourse.bass as bass
import concourse.mybir as mybir
import concourse.tile as tile
from concourse import bacc
from concourse.bass_utils import run_bass_kernel_spmd

F32 = mybir.dt.float32
BF16 = mybir.dt.bfloat16
AF = mybir.ActivationFunctionType

B = 4            # batch
N = 2048         # sequence length
D = 1024         # model dim
H = 16           # total heads
HD = 64          # head dim
HL = 8           # heads per core (local)
SCALE = HD ** -0.5

NKT = N // 128   # 16 key tiles
NDT = D // 128   # 8 d tiles
QW = 512         # q window (one attention unit)
NQW = N // QW    # 4


def _build_nc(rep=1):
    nc = bacc.Bacc(None, target_bir_lowering=False)

    xt = nc.declare_dram_parameter("xt", [D, N], BF16, isOutput=False)
    wqk = nc.declare_dram_parameter("wqk", [D, D], BF16, isOutput=False)
    wv = nc.declare_dram_parameter("wv", [D, 512], BF16, isOutput=False)
    wp = nc.declare_dram_parameter("wp", [512, D], BF16, isOutput=False)
    y = nc.declare_dram_parameter("y", [N, D], BF16, isOutput=True)

    with tile.TileContext(nc) as tc:
        for _rep in range(rep):
         with tc.tile_pool(name="ares", bufs=1) as ares, \
              tc.tile_pool(name="qkres", bufs=1) as qkres, \
              tc.tile_pool(name="vres", bufs=1) as vres, \
              tc.tile_pool(name="ores", bufs=1) as ores, \
              tc.tile_pool(name="pst", bufs=4) as pstp, \
              tc.tile_pool(name="epi", bufs=2) as epi, \
              tc.tile_pool(name="ytp", bufs=2) as ytp, \
              tc.tile_pool(name="s_ps", bufs=2, space="PSUM") as spsp, \
              tc.tile_pool(name="o_ps", bufs=2, space="PSUM") as opsp, \
              tc.tile_pool(name="bg_ps", bufs=2, space="PSUM") as bgpsp:

            # ---------------- input DMAs ----------------
            wqk_sb = [ares.tile([128, D], BF16, tag=f"wqk{dt}", name=f"wqk{dt}")
                      for dt in range(NDT)]
            xt_sb = [ares.tile([128, N], BF16, tag=f"xt{dt}", name=f"xt{dt}")
                     for dt in range(NDT)]
            wv_sb = [ares.tile([128, 512], BF16, tag=f"wv{dt}", name=f"wv{dt}")
                     for dt in range(NDT)]
            wp_sb = [ares.tile([128, D], BF16, tag=f"wp{p}", name=f"wp{p}")
                     for p in range(4)]

            # pair-0 weight columns + first x window land first so qk0 can
            # start ~5us in; the rest streams behind.  Input loads alternate
            # between the two HWDGE queues (SP + Activation) — ACT is idle
            # during the lead-in, so its queue is free bandwidth.
            _dmai = [0]

            def in_dma(out, in_):
                # ACT's queue only for the first (gating) loads — later ones
                # would steal ACT engine time from exp.
                q = nc.scalar if (_dmai[0] % 2 == 0 and _dmai[0] < 24) else nc.sync
                _dmai[0] += 1
                q.dma_start(out=out, in_=in_)

            for dt in range(NDT):
                in_dma(wqk_sb[dt][:, 512:640],
                       wqk[dt * 128:(dt + 1) * 128, 512:640])
                in_dma(xt_sb[dt][:, 0:512],
                       xt[dt * 128:(dt + 1) * 128, 0:512])
            for dt in range(NDT):
                in_dma(wqk_sb[dt][:, 0:128],
                       wqk[dt * 128:(dt + 1) * 128, 0:128])
            for dt in range(NDT):
                in_dma(wv_sb[dt], wv[dt * 128:(dt + 1) * 128, :])
            for rw in range(1, 4):
                for dt in range(NDT):
                    in_dma(xt_sb[dt][:, rw * 512:(rw + 1) * 512],
                           xt[dt * 128:(dt + 1) * 128, rw * 512:(rw + 1) * 512])
            for dt in range(NDT):
                in_dma(wqk_sb[dt][:, 128:512],
                       wqk[dt * 128:(dt + 1) * 128, 128:512])
                in_dma(wqk_sb[dt][:, 640:1024],
                       wqk[dt * 128:(dt + 1) * 128, 640:1024])
            for p in range(4):
                in_dma(wp_sb[p], wp[p * 128:(p + 1) * 128, :])

            # ---------------- resident tensors ----------------
            qres = [qkres.tile([128, N], BF16, tag=f"qr{hp}", name=f"qr{hp}")
                    for hp in range(4)]
            kres = [qkres.tile([128, N], BF16, tag=f"kr{hp}", name=f"kr{hp}")
                    for hp in range(4)]
            qodd = [qkres.tile([64, N], BF16, tag=f"qo{hp}", name=f"qo{hp}")
                    for hp in range(4)]
            kodd = [qkres.tile([64, N], BF16, tag=f"ko{hp}", name=f"ko{hp}")
                    for hp in range(4)]
            # v_aug[kt]: [128 keys, HL*(HD+1)]; per head 64 v cols + ones col
            v_aug = [vres.tile([128, HL * (HD + 1)], BF16, tag=f"va{kt}",
                               name=f"va{kt}") for kt in range(NKT)]
            ostack = [ores.tile([128, N], BF16, tag=f"os{p}", name=f"os{p}")
                      for p in range(4)]

            # ---------------- emit helpers ----------------
            def qk_window(sec, hp, rw):
                ps = bgpsp.tile([128, 512], F32, tag="bgps")
                col0 = sec * 512 + hp * 128
                for dt in range(NDT):
                    nc.tensor.matmul(
                        ps, wqk_sb[dt][:, col0:col0 + 128],
                        xt_sb[dt][:, rw * 512:(rw + 1) * 512],
                        start=(dt == 0), stop=(dt == NDT - 1))
                dst = (kres if sec else qres)[hp]
                nc.vector.tensor_copy(dst[:, rw * 512:(rw + 1) * 512], ps)

            def rebase_k(hp):
                nc.sync.dma_start(out=kodd[hp], in_=kres[hp][64:128, :])

            def rebase_q_w(hp, rw):
                sl = slice(rw * 512, (rw + 1) * 512)
                nc.sync.dma_start(out=qodd[hp][:, sl],
                                  in_=qres[hp][64:128, sl])

            def v_kt(kt):
                ps = bgpsp.tile([128, 512], F32, tag="bgps")
                for dt in range(NDT):
                    nc.tensor.matmul(
                        ps, xt_sb[dt][:, kt * 128:(kt + 1) * 128], wv_sb[dt],
                        start=(dt == 0), stop=(dt == NDT - 1))
                va3 = v_aug[kt].rearrange("p (h c) -> p h c", h=HL)
                nc.vector.tensor_copy(
                    va3[:, :, 0:HD], ps.rearrange("p (h c) -> p h c", h=HL))
                nc.vector.memset(va3[:, :, HD:HD + 1], 1.0)

            # background queue: (deadline_unit, cycles, fn), deadline-ordered.
            # unit index = qw*8 + hp*2 + ho.  k windows + the matching qw0
            # q-window must land before a pair's first unit; odd-head rebases
            # one unit later; q windows for later qw blocks before those
            # blocks.  Draining is LAZY (token bucket) so the PE backlog
            # never empties mid-attention; deadlines are force-drained at
            # unit boundaries.
            bg = []
            for hp in range(1, 4):
                d = 2 * hp
                for rw in range(4):
                    bg.append((d, 4300, lambda hp=hp, rw=rw: qk_window(1, hp, rw)))
                bg.append((d, 4300, lambda hp=hp: qk_window(0, hp, 0)))
                bg.append((d + 1, 200, lambda hp=hp: rebase_k(hp)))
                bg.append((d + 1, 100, lambda hp=hp: rebase_q_w(hp, 0)))
            for rw in range(1, 4):
                for hp in range(4):
                    d = 8 * rw + 2 * hp
                    bg.append((d, 4300, lambda hp=hp, rw=rw: qk_window(0, hp, rw)))
                    bg.append((d + 1, 100, lambda hp=hp, rw=rw: rebase_q_w(hp, rw)))
            bg.sort(key=lambda it: it[0])

            _credit = [0]

            def drain_bg(budget):
                _credit[0] += budget
                while bg and _credit[0] >= bg[0][1]:
                    _, cost, fn = bg.pop(0)
                    fn()
                    _credit[0] -= cost

            def drain_until(unit_idx):
                while bg and bg[0][0] <= unit_idx:
                    _, _, fn = bg.pop(0)
                    fn()

            # ---------------- lead: minimal prologue ----------------
            # just enough for unit (qw0, h0) strip 0: k cols 0:512 and the
            # first q window; the rest of pair-0 streams into unit 0.
            qk_window(1, 0, 0)
            qk_window(0, 0, 0)
            rebase_q_w(0, 0)
            v_kt(0)
            v_kt(1)

            # ---------------- projection helper (fed into bg) ----------------
            def proj_rt(rt):
                yt = ytp.tile([128, D], BF16, tag="yt")
                for ncol in range(2):
                    ps = bgpsp.tile([128, 512], F32, tag="bgps")
                    for p in range(4):
                        nc.tensor.matmul(
                            ps, ostack[p][:, rt * 128:(rt + 1) * 128],
                            wp_sb[p][:, ncol * 512:(ncol + 1) * 512],
                            start=(p == 0), stop=(p == 3))
                    nc.vector.tensor_copy(yt[:, ncol * 512:(ncol + 1) * 512], ps)
                nc.sync.dma_start(out=y[rt * 128:(rt + 1) * 128, :], in_=yt)

            # ---------------- attention units (qw-major) ----------------
            # qw-major ordering lets each finished qw column's projection
            # rows drain into the next block's background slack.  PV batches
            # and unit epilogues run one strip / one unit late, so the PE
            # never stalls on the freshest exp and the ACT engine never
            # stalls at unit boundaries.
            pv_pending = [None]
            tail_pending = [None]

            def flush_pv():
                if pv_pending[0] is not None:
                    pv_pending[0]()
                    pv_pending[0] = None

            def flush_tail():
                if tail_pending[0] is not None:
                    tail_pending[0]()
                    tail_pending[0] = None

            def mk_pv(s, p_sb, h, o_ps):
                def emit():
                    for i in range(2):
                        kt = 2 * s + i
                        for j in range(4):
                            # one start=True zeroes the whole 2KB PSUM
                            # zero-region (all four j sections); the other
                            # groups just accumulate.
                            nc.tensor.matmul(
                                o_ps[:, j * 65:(j + 1) * 65],
                                p_sb[:, i * 512 + j * 128:
                                     i * 512 + (j + 1) * 128],
                                v_aug[kt][:, h * (HD + 1):
                                          (h + 1) * (HD + 1)],
                                start=(kt == 0 and j == 0),
                                stop=(kt == NKT - 1 and j == 3))
                return emit

            def mk_tail(o_ps, nrm, hp, qw, ho):
                def emit():
                    # normalize rows by the per-partition ones-column sums
                    o_raw = epi.tile([128, 4 * (HD + 1)], F32, tag="oraw")
                    nc.vector.tensor_copy(o_raw, o_ps)
                    o3 = o_raw.rearrange("p (j c) -> p j c", j=4)
                    rden4 = epi.tile([128, 4], F32, tag="rden4")
                    nc.vector.reciprocal(rden4, o3[:, :, HD:HD + 1])
                    for j in range(4):
                        nc.vector.tensor_scalar_mul(
                            nrm[j][:, ho * 64:(ho + 1) * 64],
                            o3[:, j, 0:HD], rden4[:, j:j + 1])
                    if ho == 1:
                        # pair complete: transpose [q, hd] -> [hd, q] via
                        # the DMA XBAR straight into pair-stacked ostack
                        for j in range(4):
                            nc.sync.dma_start(
                                out=ostack[hp][:, qw * QW + j * 128:
                                               qw * QW + (j + 1) * 128],
                                in_=nrm[j], transpose=True)
                return emit

            nrm = None
            for qw in range(NQW):
              for hp in range(4):
                for ho in range(2):
                    h = 2 * hp + ho
                    drain_until(qw * 8 + hp * 2 + ho)
                    if ho == 0:
                        # per-(pair, qw) staging for the transposed epilogue:
                        # heads fill columns 0:64 / 64:128 of nrm
                        nrm = [epi.tile([128, 128], BF16, tag=f"nrm{j}",
                                        name=f"nrm{j}_{hp}_{qw}")
                               for j in range(4)]
                    q_t = qodd[hp] if ho else qres[hp][0:64, :]
                    k_t = kodd[hp] if ho else kres[hp][0:64, :]
                    first_unit = (h == 0 and qw == 0)
                    # swapped PV: p chunks are the stationary operand, v the
                    # moving one; out is [128 q, 65] so the softmax denom is
                    # a per-partition scalar (no broadcast needed).
                    o_ps = opsp.tile([128, 4 * (HD + 1)], F32, tag="ops")
                    q_ap = q_t[:, qw * QW:(qw + 1) * QW]
                    for s in range(8):
                        sp = spsp.tile([128, 1024], F32, tag="sps")
                        for i in range(2):
                            kt = 2 * s + i
                            nc.tensor.matmul(
                                sp[:, i * 512:(i + 1) * 512],
                                k_t[:, kt * 128:(kt + 1) * 128], q_ap,
                                start=True, stop=True)
                        p_sb = pstp.tile([128, 1024], BF16, tag="pst")
                        nc.scalar.activation(p_sb, sp, AF.Exp, scale=SCALE)
                        if first_unit:
                            # stream the rest of pair-0 k + the v tiles into
                            # unit 0's strips, just ahead of their use
                            if s < 3:
                                qk_window(1, 0, s + 1)
                            elif s == 3:
                                rebase_k(0)
                            for kt in (2 * s + 2, 2 * s + 3):
                                if kt < NKT:
                                    v_kt(kt)
                        flush_pv()
                        flush_tail()
                        pv_pending[0] = mk_pv(s, p_sb, h, o_ps)
                        if not first_unit:
                            drain_bg(500)
                    tail_pending[0] = mk_tail(o_ps, nrm, hp, qw, ho)
              # this qw column is complete for all heads: its projection
              # row-tiles become background work for the next block
              for rt in range(4 * qw, 4 * qw + 4):
                  bg.append((99, 4400, lambda rt=rt: proj_rt(rt)))

            flush_pv()
            flush_tail()
            while bg:
                _, _, fn = bg.pop(0)
                fn()
    nc.compile()
    return nc


_NC_CACHE = {}


def _get_nc(rep=1):
    if rep not in _NC_CACHE:
        _NC_CACHE[rep] = _build_nc(rep)
    return _NC_CACHE[rep]


def _run(in_maps):
    nc = _get_nc()
    return run_bass_kernel_spmd(nc, in_maps, core_ids=list(range(8)))


def _make_in_maps(x, w_qkv, w_proj):
    import ml_dtypes
    bf = ml_dtypes.bfloat16
    x = np.ascontiguousarray(x, dtype=np.float32)
    w_qkv = np.ascontiguousarray(w_qkv, dtype=np.float32)
    w_proj = np.ascontiguousarray(w_proj, dtype=np.float32)
    in_maps = []
    for c in range(8):
        b, g = divmod(c, 2)
        wq = w_qkv[:, g * 512:(g + 1) * 512]
        wk = w_qkv[:, D + g * 512:D + (g + 1) * 512]
        wvs = w_qkv[:, 2 * D + g * 512:2 * D + (g + 1) * 512]
        in_maps.append({
            "xt": np.ascontiguousarray(x[b].T).astype(bf),
            "wqk": np.ascontiguousarray(
                np.concatenate([wq, wk], axis=1)).astype(bf),
            "wv": np.ascontiguousarray(wvs).astype(bf),
            "wp": np.ascontiguousarray(w_proj[g * 512:(g + 1) * 512, :]).astype(bf),
        })
    return in_maps


def kernel(x, w_qkv, w_proj, b_proj):
    in_maps = _make_in_maps(x, w_qkv, w_proj)
    res = _run(in_maps)
    out = np.empty((B, N, D), dtype=np.float32)
    bp = np.asarray(b_proj, dtype=np.float32)
    for b in range(B):
        out[b] = (res.results[2 * b]["y"].astype(np.float32)
                  + res.results[2 * b + 1]["y"].astype(np.float32) + bp)
    return out


if __name__ == "__main__":
    rng = np.random.default_rng(0)
    x = rng.standard_normal((B, N, D), dtype=np.float32)
    w_qkv = (rng.standard_normal((D, 3 * D), dtype=np.float32) * D ** -0.5)
    w_proj = (rng.standard_normal((D, D), dtype=np.float32) * D ** -0.5)
    b_proj = np.zeros(D, dtype=np.float32)
    out = kernel(x, w_qkv, w_proj, b_proj)
    print("ran; out shape", out.shape, "mean abs", np.abs(out).mean())
